# revision 29
# baseline (speedup 1.0000x reference)
"""Trainium2 Bass kernel for 2-layer GRU (H=4, IN=6, T=512) + LeakyReLU/BN/FC head.

Strategy: pure data-parallel over batch (8 cores x 2048 rows). Within a core,
batch 2048 = 16 groups x 128; the tiny per-group GRU matmuls are packed as
block-diagonal 128-wide matmuls on the PE (bf16 operands, fp32 PSUM), gate
elementwise math runs on ACT (sigmoid/tanh w/ per-partition bias) and DVE
(fused scalar_tensor_tensor + tensor_tensor ops). The sequential T=512 scan is
fully unrolled; layer-1 step t overlaps layer-0 step t+1 across engines.
"""
import sys
import os

sys.path.insert(0, "/opt/trn_rl_repo")

import numpy as np
import ml_dtypes

import concourse.bass as bass
import concourse.bacc as bacc
import concourse.mybir as mybir
from concourse.tile import TileContext
from concourse.tile_rust import add_dep_helper

F32 = mybir.dt.float32
BF16 = mybir.dt.bfloat16
AF = mybir.ActivationFunctionType
OP = mybir.AluOpType

H = 4
IN = 6
T = 512
B = 16384
N_CORES = 8
BC = B // N_CORES          # 2048 batch per core
G = 16                     # groups per core
F = BC // G                # 128 batch per group (free dim)
CH = 8                     # timesteps per x DMA chunk
NCH = T // CH
BN_EPS = 1e-5
LEAKY_SLOPE = 0.2

# wpack column offsets (bf16 [128, 704])
C_W1RZ = 0      # [0:128, 0:128]   W1_rz   (rows 0:64 h1-side, 64:128 h0-side)
C_W0HRZ = 128   # [64:128, :]      W0h_rz
C_W0XRZ = 256   # [0:97, :]        W0x_rz   (row 96 = ones-row sigmoid bias)
C_W0XNAI = 384  # [0:97, 0:128]    W0x_nai  (cols 0:64 W0x_n, 64:128 zero;
#                                   row 96 = [b_ihn0 | b_hhn0] bias row)
C_W0HN = 512    # [64:128, 0:128]  [0 | W0h_n] (M=128 to close accum group)
C_W1NAI = 640   # [0:128, 0:128]   antidiag: [[0, W1h_n],[W1x_n, 0]]
WCOLS = 768

_cached = {}


def _block_diag_lhsT(w_gate, k_per_g, m_per_g, k_rows, m_cols):
    """lhsT[k_per_g*g + k, m_per_g*g + j] = w_gate[j, k] for each group g."""
    out = np.zeros((k_rows, m_cols), np.float32)
    for g in range(G):
        out[k_per_g * g:k_per_g * g + k_per_g,
            m_per_g * g:m_per_g * g + m_per_g] = w_gate.T
    return out


def _prep_params(p):
    """Pack weights/biases into device layouts (replicated across cores)."""
    w_ih0, w_hh0 = p["w_ih0"], p["w_hh0"]
    w_ih1, w_hh1 = p["w_ih1"], p["w_hh1"]

    wpack = np.zeros((128, WCOLS), np.float32)
    # Layer-0 rz tile rows: [r(0:64); z(64:128)].
    # Layer-1 rz tile rows SWAPPED: [z(0:64); r(64:128)] (so that the z-half
    # sits at base partition 0, matching d1/h1 for the walrus same-base rule).
    # W1_rz: K rows 0:64 = h1 (w_hh1), 64:128 = h0 (w_ih1)
    for gate in range(2):
        rows = slice(gate * H, gate * H + H)
        g1col = C_W1RZ + 64 * (1 - gate)   # layer1: z first, r second
        wpack[0:64, g1col:g1col + 64] = \
            _block_diag_lhsT(w_hh1[rows], H, H, 64, 64)
        wpack[64:128, g1col:g1col + 64] = \
            _block_diag_lhsT(w_ih1[rows], H, H, 64, 64)
        wpack[64:128, C_W0HRZ + 64 * gate:C_W0HRZ + 64 * gate + 64] = \
            _block_diag_lhsT(w_hh0[rows], H, H, 64, 64)
        wpack[0:96, C_W0XRZ + 64 * gate:C_W0XRZ + 64 * gate + 64] = \
            _block_diag_lhsT(w_ih0[rows], IN, H, 96, 64)
    nrows = slice(2 * H, 3 * H)
    wpack[0:96, C_W0XNAI:C_W0XNAI + 64] = \
        _block_diag_lhsT(w_ih0[nrows], IN, H, 96, 64)
    wpack[64:128, C_W0HN + 64:C_W0HN + 128] = \
        _block_diag_lhsT(w_hh0[nrows], H, H, 64, 64)
    # L1 n-tile in ONE matmul over rhs=[h1; h0]:  [[0, W1h_n], [W1x_n, 0]]
    wpack[64:128, C_W1NAI:C_W1NAI + 64] = _block_diag_lhsT(w_ih1[nrows], H, H, 64, 64)
    wpack[0:64, C_W1NAI + 64:C_W1NAI + 128] = \
        _block_diag_lhsT(w_hh1[nrows], H, H, 64, 64)
    # L0 bias ones-rows (x tile row 96 is constant 1.0):
    # sigma0 bias on W0x_rz row 96; b_ihn0 / b_hhn0 on W0x_nai row 96.
    for gate in range(2):
        for g in range(G):
            for j in range(H):
                wpack[96, C_W0XRZ + 64 * gate + 4 * g + j] = \
                    p["b_ih0"][gate * H + j] + p["b_hh0"][gate * H + j]
    for g in range(G):
        for j in range(H):
            wpack[96, C_W0XNAI + 4 * g + j] = p["b_ih0"][2 * H + j]
            wpack[96, C_W0XNAI + 64 + 4 * g + j] = p["b_hh0"][2 * H + j]
    wpack = wpack.astype(ml_dtypes.bfloat16)

    # biasv [128, 8] f32.  Row placement must match where each op consumes it
    # (walrus requires equal SBUF base partitions for paired SB inputs):
    #  col0: sigmoid bias L0 [r(0:64); z(64:128)]
    #  col1: sigmoid bias L1 [z(0:64); r(64:128)]
    #  col2: b_ihn0 rows 0:64   col3: b_hhn0 rows 0:64
    #  col4: b_ihn1 rows 64:128 col5: b_hhn1 rows 64:128
    biasv = np.zeros((128, 8), np.float32)
    for layer, (bi, bh) in enumerate([(p["b_ih0"], p["b_hh0"]),
                                      (p["b_ih1"], p["b_hh1"])]):
        col = np.zeros(128, np.float32)
        for gate in range(2):
            half = gate if layer == 0 else (1 - gate)
            for g in range(G):
                for j in range(H):
                    col[64 * half + 4 * g + j] = bi[gate * H + j] + bh[gate * H + j]
        biasv[:, layer] = col
        for g in range(G):
            for j in range(H):
                biasv[64 * layer + 4 * g + j, 2 + 2 * layer] = bi[2 * H + j]
                biasv[64 + 4 * g + j, 3 + 2 * layer] = bh[2 * H + j]
    # BN affine folded: scale = gamma*rsqrt(var+eps); shift = beta - mean*scale
    scale = p["bn_gamma"] / np.sqrt(p["bn_var"] + BN_EPS)
    shift = p["bn_beta"] - p["bn_mean"] * scale
    for g in range(G):
        for j in range(H):
            biasv[4 * g + j, 6] = scale[j]
            biasv[4 * g + j, 7] = shift[j]

    # fc lhsT [64, 16] f32: out[g, f] = sum_j fc_w[0, j] * bn[4g+j, f]
    wfc = np.zeros((64, 16), np.float32)
    for g in range(G):
        for j in range(H):
            wfc[4 * g + j, g] = p["fc_w"][0, j]

    return wpack, biasv, wfc


def _prep_x(encoded):
    """[B, T, IN] f32 -> per-core [NCH, 97, CH*F] bf16 (block-diag rhs layout,
    row 96 = constant 1.0 for the matmul bias ones-row trick)."""
    e = encoded.reshape(N_CORES, G, F, NCH, CH, IN)      # core, g, f, c, ts, i
    xt = np.empty((N_CORES, NCH, G * IN + 1, CH * F), ml_dtypes.bfloat16)
    xt[:, :, :G * IN, :] = e.transpose(0, 3, 1, 5, 4, 2).reshape(
        N_CORES, NCH, G * IN, CH * F).astype(ml_dtypes.bfloat16)
    xt[:, :, G * IN, :] = 1.0
    return xt


def _build(t_steps=T):
    nc = bacc.Bacc("TRN2", target_bir_lowering=False, debug=False,
                   num_devices=N_CORES)
    n_ch = (t_steps + CH - 1) // CH

    xT = nc.dram_tensor("xT", [n_ch, G * IN + 1, CH * F], BF16,
                        kind="ExternalInput").ap()
    wpack_d = nc.dram_tensor("wpack", [128, WCOLS], BF16,
                             kind="ExternalInput").ap()
    biasv_d = nc.dram_tensor("biasv", [128, 8], F32, kind="ExternalInput").ap()
    wfc_d = nc.dram_tensor("wfc", [64, 16], F32, kind="ExternalInput").ap()
    concb_d = nc.dram_tensor("concb", [G, F], F32, kind="ExternalInput").ap()
    out_d = nc.dram_tensor("out", [G, F], F32, kind="ExternalOutput").ap()

    with TileContext(nc) as tc:
        with (
            tc.tile_pool(name="const", bufs=1) as constp,
            tc.tile_pool(name="xin", bufs=3) as xinp,
            tc.tile_pool(name="state", bufs=1) as statep,
            tc.tile_pool(name="rzps0", bufs=3, space="PSUM") as rzps0,
            tc.tile_pool(name="rzps1", bufs=3, space="PSUM") as rzps1,
            tc.tile_pool(name="nps0", bufs=1, space="PSUM") as nps0,
            tc.tile_pool(name="nps1", bufs=1, space="PSUM") as nps1,
            tc.tile_pool(name="work", bufs=3) as workp,
        ):
            wb = constp.tile([128, WCOLS], BF16)
            nc.sync.dma_start(wb[:, :], wpack_d[:, :])
            bv = constp.tile([128, 8], F32)
            nc.sync.dma_start(bv[:, :], biasv_d[:, :])
            wfc = constp.tile([64, 16], F32)
            nc.sync.dma_start(wfc[:, :], wfc_d[:, :])
            concb = constp.tile([G, F], F32)
            nc.sync.dma_start(concb[:, :], concb_d[:, :])

            hbuf = statep.tile([128, F], BF16)   # [0:64]=h1, [64:128]=h0
            nc.vector.memset(hbuf[:, :], 0.0)

            # ~6us of back-to-back matmuls to flip the PE HAM clock gate to
            # K=8/8 (2.4 GHz) before the scan starts.
            warm = rzps0.tile([128, F], F32, tag="rz0")
            for _ in range(56):
                nc.tensor.matmul(warm[:, :], wb[0:128, 0:128],
                                 wb[0:128, 128:128 + F], start=True, stop=True)

            xc_tiles = {}

            def get_xs(t):
                c, ts = divmod(t, CH)
                if c not in xc_tiles:
                    xc = xinp.tile([G * IN + 1, CH * F], BF16, tag="xc")
                    nc.sync.dma_start(xc[:, :], xT[c, :, :])
                    xc_tiles[c] = xc
                return xc_tiles[c][:, ts * F:(ts + 1) * F]

            def emit_l0(t):
                xs = get_xs(t)
                # x-only matmuls first (can run ahead); biases ride the
                # ones-row (row 96 of the x tile).
                rz0 = rzps0.tile([128, F], F32, tag="rz0")
                n0 = nps0.tile([128, F], F32, tag="n0")
                nc.tensor.matmul(rz0[:, :], wb[0:97, C_W0XRZ:C_W0XRZ + 128],
                                 xs, start=True, stop=False)
                nc.tensor.matmul(n0[:, :], wb[0:97, C_W0XNAI:C_W0XNAI + 128],
                                 xs, start=True, stop=False)
                nc.tensor.matmul(rz0[:, :], wb[64:128, C_W0HRZ:C_W0HRZ + 128],
                                 hbuf[64:128, :], start=False, stop=True)
                mm_ah0 = nc.tensor.matmul(
                    n0[:, :], wb[64:128, C_W0HN:C_W0HN + 128],
                    hbuf[64:128, :], start=False, stop=True)

                # L0 gate rows: r=[0:64], z=[64:128]; h0 lives at base 64,
                # so n/d/e tiles sit at base-64 slices of [128,F] tiles.
                rzs0 = workp.tile([128, F], BF16, tag="rzs")
                sig0 = nc.scalar.activation(rzs0[:, :], rz0[:, :], AF.Sigmoid)
                m0 = workp.tile([64, F], BF16, tag="m")
                nc.vector.tensor_mul(m0[:, :], n0[64:128, :], rzs0[0:64, :])
                an0 = workp.tile([64, F], F32, tag="an")
                add0 = nc.vector.tensor_add(an0[:, :], n0[0:64, :], m0[:, :])
                nn0 = workp.tile([128, F], BF16, tag="nn")
                th0 = nc.scalar.activation(nn0[64:128, :], an0[:, :], AF.Tanh)
                d0 = workp.tile([128, F], BF16, tag="d")
                nc.vector.tensor_sub(d0[64:128, :], hbuf[64:128, :],
                                     nn0[64:128, :])
                e0 = workp.tile([128, F], BF16, tag="e")
                nc.vector.tensor_mul(e0[64:128, :], rzs0[64:128, :],
                                     d0[64:128, :])
                nc.vector.tensor_add(hbuf[64:128, :], nn0[64:128, :],
                                     e0[64:128, :])
                return {"ah": mm_ah0, "sig": sig0, "an": add0, "tanh": th0}

            def emit_l1(t):
                rz1 = rzps1.tile([128, F], F32, tag="rz1")
                mm_rz1 = nc.tensor.matmul(
                    rz1[:, :], wb[0:128, C_W1RZ:C_W1RZ + 128],
                    hbuf[0:128, :], start=True, stop=True)
                n1 = nps1.tile([128, F], F32, tag="n1")
                nc.tensor.matmul(n1[:, :], wb[0:128, C_W1NAI:C_W1NAI + 128],
                                 hbuf[0:128, :], start=True, stop=True)

                # L1 gate rows SWAPPED: z=[0:64], r=[64:128]; h1 lives at
                # base 0, so m/an sit at base-64 slices, n/d/e at base 0.
                rzs1 = workp.tile([128, F], BF16, tag="rzs")
                sig1 = nc.scalar.activation(rzs1[:, :], rz1[:, :], AF.Sigmoid,
                                            bias=bv[:, 1:2])
                m1 = workp.tile([128, F], BF16, tag="m")
                stt1 = nc.vector.scalar_tensor_tensor(
                    m1[64:128, :], n1[64:128, :], bv[64:128, 5:6],
                    rzs1[64:128, :], op0=OP.add, op1=OP.mult)
                an1 = workp.tile([128, F], F32, tag="an")
                nc.vector.tensor_add(an1[64:128, :], n1[0:64, :],
                                     m1[64:128, :])
                nn1 = workp.tile([64, F], BF16, tag="nn")
                th1 = nc.scalar.activation(nn1[:, :], an1[64:128, :], AF.Tanh,
                                           bias=bv[64:128, 4:5])
                d1 = workp.tile([64, F], BF16, tag="d")
                nc.vector.tensor_sub(d1[:, :], hbuf[0:64, :], nn1[:, :])
                e1 = workp.tile([64, F], BF16, tag="e")
                nc.vector.tensor_mul(e1[:, :], rzs1[0:64, :], d1[:, :])
                nc.vector.tensor_add(hbuf[0:64, :], nn1[:, :], e1[:, :])
                return {"rz1": mm_rz1, "sig": sig1, "m": stt1, "tanh": th1}

            # Software-pipelined emission: body tau emits L1(tau-1) then
            # L0(tau), matching steady-state readiness so per-engine FIFO
            # order doesn't jam the critical layer-0 recurrence chain.
            emit_l0(0)
            for tau in range(1, t_steps):
                emit_l1(tau - 1)
                emit_l0(tau)
                # Steer per-engine FIFO order to match steady-state readiness:
                # L0(tau)'s chain ops ahead of L1(tau-1)'s on PE/ACT/DVE.
            emit_l1(t_steps - 1)

            # ---- epilogue: LeakyReLU -> BN affine -> FC(+conc) ----
            lr = workp.tile([64, F], F32, tag="lr")
            nc.vector.scalar_tensor_tensor(lr[:, :], hbuf[0:64, :], LEAKY_SLOPE,
                                           hbuf[0:64, :], op0=OP.mult, op1=OP.max)
            bn = workp.tile([64, F], F32, tag="bn")
            nc.vector.tensor_scalar(bn[:, :], lr[:, :], bv[0:64, 6:7],
                                    bv[0:64, 7:8], op0=OP.mult, op1=OP.add)
            fc = rzps0.tile([16, F], F32, tag="rz0")
            nc.tensor.matmul(fc[:, :], wfc[:, :], bn[:, :], start=True, stop=True)
            outs = workp.tile([16, F], F32, tag="outs")
            nc.vector.scalar_tensor_tensor(outs[:, :], concb[:, :], 1.0, fc[:, :],
                                           op0=OP.mult, op1=OP.add)
            nc.sync.dma_start(out_d[:, :], outs[:, :])

    nc.compile()
    return nc


last_exec_time_ns = None


def kernel(**inputs):
    from concourse.bass_utils import run_bass_kernel_spmd

    key = "prog"
    if key not in _cached:
        _cached[key] = _build(T)
    nc = _cached[key]

    wpack, biasv, wfc = _prep_params(inputs)
    xt = _prep_x(np.asarray(inputs["encoded"], np.float32))
    conc = np.asarray(inputs["conc"], np.float32)
    fc4 = float(inputs["fc_w"][0, H])
    fcb = float(inputs["fc_b"][0])
    concb = (fc4 * conc + fcb).reshape(N_CORES, G, F)

    in_maps = []
    for i in range(N_CORES):
        in_maps.append({
            "xT": xt[i],
            "wpack": wpack,
            "biasv": biasv,
            "wfc": wfc,
            "concb": concb[i],
        })
    trace = bool(os.environ.get("KERNEL_TRACE"))
    kwargs = {}
    if trace:
        import types, tempfile
        from trn_agent_boot.trn_boot import _ntff_profile_via_ctypes
        _m = types.ModuleType('antenv.axon_hooks')
        _m.get_axon_ntff_profile_hook = \
            lambda: _ntff_profile_via_ctypes('/opt/axon/libaxon_pjrt.so')
        sys.modules['antenv.axon_hooks'] = _m
        kwargs = dict(trace=True, tmpdir=tempfile.mkdtemp())
    res = run_bass_kernel_spmd(nc, in_maps, list(range(N_CORES)), **kwargs)
    if trace:
        global last_exec_time_ns
        last_exec_time_ns = res.exec_time_ns
    outs = [res.results[i]["out"].reshape(BC) for i in range(N_CORES)]
    return np.concatenate(outs).reshape(B, 1).astype(np.float32)
